# revision 36
# baseline (speedup 1.0000x reference)
"""FP8 semi-sparse (2:4) activation linear — Trainium2 Bass/Tile kernel, v2.

Reference semantics:
  Wq, W_scale = rowwise fp8(e4m3fn) quant of weight      [N, K]
  Xq, X_scale = rowwise fp8(e4m3fn) quant of x           [M, K]
  Xsp         = 2:4 sparsify of Xq (keep 2 largest |.| per group of 4,
                ties -> earlier index)
  out         = (Xsp @ Wq^T) * X_scale * W_scale^T  -> bf16

v2 design (vs the v1 data-parallel kernel):
  * 2D core grid 4x2: core c=(mg*2+ng) gets x rows [mg*2048,+2048) and W rows
    [ng*2048,+2048), computes the [2048, 2048] out block.  Halves the
    replicated W work and the per-core HBM traffic vs pure DP.
  * W^T fp8 (deinterleaved DoubleRow planes) is SBUF-resident (64KB/part);
    quantized in 4 bands of 512 rows so matmuls can start after band 0.
  * m-outer software pipeline: X-path(m) on DVE runs ahead while PE drains
    matmul pairs (m, ns); keeps PE continuously busy so it p-state ramps.
  * Engine balance: DVE owns the 2:4 selection (+half of each X amax);
    ACT owns the fp8 casts + deinterleaves + psum->bf16 epilogue cast;
    Pool(gpsimd) owns W amax folding, X amax half-folds, and the bf16
    epilogue scale-by-W_amax multiply.
  * Transposes are single big XBAR instructions ([rows, 2048]u16 ->
    [128, 16, rows]) straight from DRAM staging, 3D-out form.
  * TRN fp8e4 (max 240) vs OCP e4m3fn (max 448): quantize at HALF scale
    (g = 224/amax), fold the 4x into the output scale constant.
"""

import numpy as np

import concourse.bass as bass
import concourse.mybir as mybir
import concourse.tile as tile
from concourse import bacc
from concourse.bass_utils import run_bass_kernel_spmd

P = 128
M_FULL, K_FULL, N_FULL = 8192, 4096, 4096
NCORES = 8
MG, NG = 4, 2
M_CORE = M_FULL // MG    # 2048
N_CORE = N_FULL // NG    # 2048
N_SLICE = 512

F32 = mybir.dt.float32
BF16 = mybir.dt.bfloat16
FP8 = mybir.dt.float8e4
U16 = mybir.dt.uint16

AX = mybir.AxisListType.X
OP = mybir.AluOpType
AF = mybir.ActivationFunctionType

# out = acc' * amax_w * (amax_x * 4/448^2); acc' is the matmul of halved values
SX_CONST = float(np.float32(4.0 / (448.0 * 448.0)))


U32 = mybir.dt.uint32


def build_nc(m_core=M_CORE, k=K_FULL, n=N_CORE) -> bass.Bass:
    assert m_core % P == 0 and k % (2 * P) == 0 and n % N_SLICE == 0
    m_tiles = m_core // P          # 16
    kp_tiles = k // (2 * P)        # 16 packed k-pair tiles
    n_slices = n // N_SLICE        # 4 (also W bands)
    w_tiles = n // P               # 16
    wt_per_band = w_tiles // n_slices  # 4
    groups = k // 4
    kh = k // 2                    # half-row length (f32 load halves)

    nc = bacc.Bacc()
    x = nc.declare_dram_parameter("x", [m_core, k], F32, isOutput=False)
    w = nc.declare_dram_parameter("weight", [n, k], F32, isOutput=False)
    out = nc.declare_dram_parameter("out", [m_core, n], BF16, isOutput=True)

    with tile.TileContext(nc) as tc:
        with (
            tc.tile_pool(name="dram", bufs=1, space="DRAM") as dpool,
            tc.tile_pool(name="per", bufs=1) as perpool,
            tc.tile_pool(name="wld", bufs=2) as wldpool,
            tc.tile_pool(name="wq8", bufs=1) as wqpool,
            tc.tile_pool(name="wtp", bufs=1) as wtppool,
            tc.tile_pool(name="xld", bufs=2) as xldpool,
            tc.tile_pool(name="xu8", bufs=2) as u8pool,
            tc.tile_pool(name="cmp", bufs=1) as cpool,
            tc.tile_pool(name="xsT0", bufs=3) as xspT0pool,
            tc.tile_pool(name="xsT", bufs=2) as xspTpool,
            tc.tile_pool(name="sml", bufs=4) as spool,
            tc.tile_pool(name="ep", bufs=2) as eppool,
            tc.tile_pool(name="ps", bufs=8, space="PSUM") as pspool,
        ):
            xsp_dram = dpool.tile([m_core, k], FP8)
            wq_dram = dpool.tile([n, k], FP8)
            wamax_dram = dpool.tile([n], F32)

            # persistent SBUF
            wk2 = perpool.tile([P, kp_tiles, 2, n], FP8)       # 64KB/part
            swb = perpool.tile([P, n], BF16)                   # 4KB/part
            sx4 = perpool.tile([P, m_tiles], F32)

            xspT = {}  # m -> [P, kp_tiles, 2, P] fp8 tile

            # ---------------- W path ----------------
            def w_quant(j):
                h0 = wldpool.tile([P, kh], F32, tag="wh0")
                nc.sync.dma_start(h0, w[P * j : P * (j + 1), :kh])
                h1 = wldpool.tile([P, kh], F32, tag="wh1")
                nc.sync.dma_start(h1, w[P * j : P * (j + 1), kh:])
                a0 = spool.tile([P, 1], F32, tag="wa0")
                nc.vector.tensor_reduce(
                    a0, h0, axis=AX, op=OP.max, apply_absolute_value=True
                )
                a1 = spool.tile([P, 1], F32, tag="wa1")
                nc.vector.tensor_reduce(
                    a1, h1, axis=AX, op=OP.max, apply_absolute_value=True
                )
                amax = spool.tile([P, 1], F32, tag="wamax")
                nc.vector.tensor_tensor(amax, a0, a1, op=OP.max)
                g = spool.tile([P, 1], F32, tag="wg")
                nc.vector.reciprocal(g, amax)
                nc.vector.tensor_scalar_mul(g, g, 224.0)
                u8 = wqpool.tile([P, k], FP8, tag="wu8")
                nc.scalar.activation(u8[:, :kh], h0, AF.Copy, scale=g)
                nc.scalar.activation(u8[:, kh:], h1, AF.Copy, scale=g)
                nc.sync.dma_start(wamax_dram[P * j : P * (j + 1)], amax)
                nc.sync.dma_start(wq_dram[P * j : P * (j + 1), :], u8)

            def w_band(ns):
                # transpose band [512, 2048]u16 -> [128, 16, 512], by k-halves
                kp2 = kp_tiles // 2
                for kb in range(2):
                    wtp = wtppool.tile([P, kp2, N_SLICE], U16, tag="wtp")
                    nc.sync.dma_start_transpose(
                        wtp,
                        wq_dram.bitcast(U16)[
                            N_SLICE * ns : N_SLICE * (ns + 1),
                            P * kp2 * kb : P * kp2 * (kb + 1),
                        ],
                    )
                    pk = wtp.bitcast(FP8).rearrange("p t (r o) -> p t r o", o=2)
                    for o in range(2):
                        nc.scalar.activation(
                            wk2[
                                :, kp2 * kb : kp2 * (kb + 1), o,
                                N_SLICE * ns : N_SLICE * (ns + 1),
                            ],
                            pk[:, :, :, o],
                            AF.Copy,
                        )
                # broadcast W amax into swb (bf16, via SWDGE cast dma)
                nc.gpsimd.dma_start(
                    swb[:, N_SLICE * ns : N_SLICE * (ns + 1)],
                    wamax_dram[N_SLICE * ns : N_SLICE * (ns + 1)]
                    .unsqueeze(0)
                    .to_broadcast([P, N_SLICE]),
                )

            # ---------------- X path ----------------
            def x_tile(mt):
                h0 = xldpool.tile([P, kh], F32, tag="xh0")
                nc.sync.dma_start(h0, x[P * mt : P * (mt + 1), :kh])
                h1 = xldpool.tile([P, kh], F32, tag="xh1")
                nc.sync.dma_start(h1, x[P * mt : P * (mt + 1), kh:])
                a0 = spool.tile([P, 1], F32, tag="xa0")
                nc.vector.tensor_reduce(
                    a0, h0, axis=AX, op=OP.max, apply_absolute_value=True
                )
                a1 = spool.tile([P, 1], F32, tag="xa1")
                nc.vector.tensor_reduce(
                    a1, h1, axis=AX, op=OP.max, apply_absolute_value=True
                )
                amax = spool.tile([P, 1], F32, tag="xamax")
                nc.vector.tensor_tensor(amax, a0, a1, op=OP.max)
                nc.vector.tensor_scalar_mul(sx4[:, mt : mt + 1], amax, SX_CONST)
                g = spool.tile([P, 1], F32, tag="xg")
                nc.vector.reciprocal(g, amax)
                nc.vector.tensor_scalar_mul(g, g, 224.0)
                u8 = u8pool.tile([P, k], FP8, tag="xu8")
                nc.scalar.activation(u8[:, :kh], h0, AF.Copy, scale=g)
                nc.scalar.activation(u8[:, kh:], h1, AF.Copy, scale=g)

                # ---- 2:4 selection in packed u16 domain (DVE) ----
                # SBUF-overlaid scratch: magmask holds mag then (later) the
                # byte-mask; scr holds mlo+mhi then (later) the kk flags;
                # b6mt holds the 6 pairwise compares then (later) mtmp;
                # stile holds the s partial sums then (later) the masked xsp.
                magmask = cpool.tile([P, k // 2], U16, tag="magmask")
                scr = cpool.tile([P, k], U16, tag="scr")
                b6mt = cpool.tile([P, 6, groups], U16, tag="b6mt")
                stile = cpool.tile([P, k // 2], U16, tag="stile")

                mag = magmask
                nc.vector.tensor_scalar(
                    mag, u8.bitcast(U16), 0x7F7F, None, op0=OP.bitwise_and
                )
                mlo = scr[:, : k // 2]
                nc.vector.tensor_scalar(mlo, mag, 0x00FF, None, op0=OP.bitwise_and)
                mhi = scr[:, k // 2 :]
                nc.vector.tensor_scalar(
                    mhi, mag, 8, None, op0=OP.logical_shift_right
                )
                lo = mlo.rearrange("p (g t) -> p g t", t=2)
                hi = mhi.rearrange("p (g t) -> p g t", t=2)
                e = {0: lo[:, :, 0], 1: hi[:, :, 0], 2: lo[:, :, 1], 3: hi[:, :, 1]}

                b6 = b6mt
                pairs = [(0, 1), (0, 2), (0, 3), (1, 2), (1, 3), (2, 3)]
                bidx = {}
                for pi, (i, jj) in enumerate(pairs):
                    nc.vector.tensor_tensor(b6[:, pi, :], e[i], e[jj], op=OP.is_ge)
                    bidx[(i, jj)] = pi

                def b(i, jj):
                    return b6[:, bidx[(i, jj)], :]

                kk = scr.bitcast(BF16).rearrange("p (a g) -> p a g", a=4)
                s = stile.bitcast(BF16).rearrange("p (a g) -> p a g", a=2)
                nc.vector.tensor_tensor(s[:, 0, :], b(0, 1), b(0, 2), op=OP.add)
                nc.vector.tensor_tensor(s[:, 0, :], s[:, 0, :], b(0, 3), op=OP.add)
                nc.vector.tensor_scalar(kk[:, 0, :], s[:, 0, :], 2.0, None, op0=OP.is_ge)
                nc.vector.tensor_tensor(s[:, 1, :], b(1, 2), b(1, 3), op=OP.add)
                nc.vector.tensor_tensor(s[:, 1, :], s[:, 1, :], b(0, 1), op=OP.subtract)
                nc.vector.tensor_scalar(kk[:, 1, :], s[:, 1, :], 1.0, None, op0=OP.is_ge)
                nc.vector.tensor_tensor(s[:, 0, :], b(2, 3), b(0, 2), op=OP.subtract)
                nc.vector.tensor_tensor(s[:, 0, :], s[:, 0, :], b(1, 2), op=OP.subtract)
                nc.vector.tensor_scalar(kk[:, 2, :], s[:, 0, :], 0.0, None, op0=OP.is_ge)
                nc.vector.tensor_tensor(s[:, 1, :], b(0, 3), b(1, 3), op=OP.add)
                nc.vector.tensor_tensor(s[:, 1, :], s[:, 1, :], b(2, 3), op=OP.add)
                nc.vector.tensor_scalar(kk[:, 3, :], s[:, 1, :], 1.0, None, op0=OP.is_le)

                mtmp = b6mt.bitcast(BF16).rearrange("p a g -> p a g")[:, :2, :]
                nc.vector.tensor_scalar_mul(mtmp[:, 0, :], kk[:, 0, :], 255.0)
                nc.vector.tensor_scalar_mul(mtmp[:, 1, :], kk[:, 2, :], 255.0)
                mask = magmask
                mv = mask.rearrange("p (g t) -> p g t", t=2)
                nc.vector.scalar_tensor_tensor(
                    mv[:, :, 0], kk[:, 1, :], 65280.0, mtmp[:, 0, :],
                    op0=OP.mult, op1=OP.add,
                )
                nc.vector.scalar_tensor_tensor(
                    mv[:, :, 1], kk[:, 3, :], 65280.0, mtmp[:, 1, :],
                    op0=OP.mult, op1=OP.add,
                )
                xsp = stile
                nc.vector.tensor_tensor(xsp, u8.bitcast(U16), mask, op=OP.bitwise_and)
                nc.sync.dma_start(
                    xsp_dram.bitcast(U16)[P * mt : P * (mt + 1), :], xsp
                )

                # transpose this m-row-block: [128, 2048]u16 -> [128, 16, 128]
                # (overlaid on scr, whose kk contents are dead by now)
                xtp = scr[:, : k // 2].rearrange("p (t r) -> p t r", t=kp_tiles)
                nc.sync.dma_start_transpose(
                    xtp, xsp_dram.bitcast(U16)[P * mt : P * (mt + 1), :]
                )
                pool_ = xspT0pool if mt < 3 else xspTpool
                xq = pool_.tile([P, kp_tiles, 2, P], FP8, tag="xq", name="xq")
                pk = xtp.bitcast(FP8).rearrange("p t (r o) -> p t r o", o=2)
                nc.gpsimd.tensor_copy(xq[:, :, 0, :], pk[:, :, :, 0])
                nc.gpsimd.tensor_copy(xq[:, :, 1, :], pk[:, :, :, 1])
                xspT[mt] = xq

            # ---------------- MM pair ----------------
            def mm_pair(mt, ns):
                ps = pspool.tile([P, N_SLICE], F32, tag="ps", name=f"ps{mt}_{ns}")
                xq = xspT[mt]
                for t in range(kp_tiles):
                    nc.tensor.matmul(
                        ps,
                        lhsT=xq[:, t, :, :],
                        rhs=wk2[:, t, :, N_SLICE * ns : N_SLICE * (ns + 1)],
                        perf_mode=mybir.MatmulPerfMode.DoubleRow,
                        start=(t == 0),
                        stop=(t == kp_tiles - 1),
                    )
                ob = eppool.tile([P, N_SLICE], BF16, tag="ob")
                nc.scalar.activation(ob, ps, AF.Copy, scale=sx4[:, mt : mt + 1])
                ob2 = eppool.tile([P, N_SLICE], BF16, tag="ob2")
                nc.gpsimd.tensor_tensor(
                    ob2, ob, swb[:, N_SLICE * ns : N_SLICE * (ns + 1)], op=OP.mult
                )
                nc.sync.dma_start(
                    out[P * mt : P * (mt + 1), N_SLICE * ns : N_SLICE * (ns + 1)],
                    ob2,
                )

            # ---------------- emission schedule ----------------
            # X0-2 first (DVE starts immediately; their xspT tiles live in a
            # dedicated pinned pool since their MM pairs are emitted last),
            # then the whole W path, then the steady-state X pipeline where
            # pairs(m, 0..3) are emitted en-bloc right after x_tile(m) — so
            # the rolling xsT pool (bufs=2) is correct by construction.
            # The PE executes by semaphore readiness, not emission position,
            # so matmuls start as soon as band 0 + one X tile are done.
            x_tile(0); x_tile(1); x_tile(2)
            for bn in range(n_slices):
                for j in range(wt_per_band * bn, wt_per_band * (bn + 1)):
                    w_quant(j)
                w_band(bn)
            for mt in range(3, m_tiles):
                x_tile(mt)
                for ns in range(n_slices):
                    mm_pair(mt, ns)
            for mt in range(3):
                for ns in range(n_slices):
                    mm_pair(mt, ns)

    return nc


_NC = None


def make_in_maps(x: np.ndarray, weight: np.ndarray) -> list[dict]:
    x = np.ascontiguousarray(x, dtype=np.float32)
    weight = np.ascontiguousarray(weight, dtype=np.float32)
    in_maps = []
    for c in range(NCORES):
        mg, ng = c // NG, c % NG
        in_maps.append(
            {
                "x": x[mg * M_CORE : (mg + 1) * M_CORE],
                "weight": weight[ng * N_CORE : (ng + 1) * N_CORE],
            }
        )
    return in_maps


def assemble_out(results: list[dict]) -> np.ndarray:
    rows = []
    for mg in range(MG):
        blocks = [results[mg * NG + ng]["out"] for ng in range(NG)]
        rows.append(np.concatenate(blocks, axis=1))
    return np.concatenate(rows, axis=0)


def kernel(x: np.ndarray, weight: np.ndarray) -> np.ndarray:
    global _NC
    if _NC is None:
        _NC = build_nc()
        _NC.finalize()
    res = run_bass_kernel_spmd(_NC, make_in_maps(x, weight), list(range(NCORES)))
    return assemble_out(res.results)


# revision 47
# speedup vs baseline: 1.1546x; 1.1546x over previous
"""FP8 semi-sparse (2:4) activation linear — Trainium2 Bass/Tile kernel, v2.

Reference semantics:
  Wq, W_scale = rowwise fp8(e4m3fn) quant of weight      [N, K]
  Xq, X_scale = rowwise fp8(e4m3fn) quant of x           [M, K]
  Xsp         = 2:4 sparsify of Xq (keep 2 largest |.| per group of 4,
                ties -> earlier index)
  out         = (Xsp @ Wq^T) * X_scale * W_scale^T  -> bf16

v2 design (vs the v1 data-parallel kernel):
  * 2D core grid 4x2: core c=(mg*2+ng) gets x rows [mg*2048,+2048) and W rows
    [ng*2048,+2048), computes the [2048, 2048] out block.  Halves the
    replicated W work and the per-core HBM traffic vs pure DP.
  * W^T fp8 (deinterleaved DoubleRow planes) is SBUF-resident (64KB/part);
    quantized in 4 bands of 512 rows so matmuls can start after band 0.
  * m-outer software pipeline: X-path(m) on DVE runs ahead while PE drains
    matmul pairs (m, ns); keeps PE continuously busy so it p-state ramps.
  * Engine balance: DVE owns the 2:4 selection (+half of each X amax);
    ACT owns the fp8 casts + deinterleaves + psum->bf16 epilogue cast;
    Pool(gpsimd) owns W amax folding, X amax half-folds, and the bf16
    epilogue scale-by-W_amax multiply.
  * Transposes are single big XBAR instructions ([rows, 2048]u16 ->
    [128, 16, rows]) straight from DRAM staging, 3D-out form.
  * TRN fp8e4 (max 240) vs OCP e4m3fn (max 448): quantize at HALF scale
    (g = 224/amax), fold the 4x into the output scale constant.
"""

import numpy as np

import concourse.bass as bass
import concourse.mybir as mybir
import concourse.tile as tile
from concourse import bacc
from concourse.bass_utils import run_bass_kernel_spmd

P = 128
M_FULL, K_FULL, N_FULL = 8192, 4096, 4096
NCORES = 8
MG, NG = 4, 2
M_CORE = M_FULL // MG    # 2048
N_CORE = N_FULL // NG    # 2048
N_SLICE = 512

F32 = mybir.dt.float32
BF16 = mybir.dt.bfloat16
FP8 = mybir.dt.float8e4
U16 = mybir.dt.uint16

AX = mybir.AxisListType.X
OP = mybir.AluOpType
AF = mybir.ActivationFunctionType

# out = acc' * amax_w * (amax_x * 4/448^2); acc' is the matmul of halved values
SX_CONST = float(np.float32(4.0 / (448.0 * 448.0)))


U32 = mybir.dt.uint32


def build_nc(m_core=M_CORE, k=K_FULL, n=N_CORE) -> bass.Bass:
    assert m_core % P == 0 and k % (2 * P) == 0 and n % N_SLICE == 0
    m_tiles = m_core // P          # 16
    kp_tiles = k // (2 * P)        # 16 packed k-pair tiles
    n_slices = n // N_SLICE        # 4 (also W bands)
    w_tiles = n // P               # 16
    wt_per_band = w_tiles // n_slices  # 4
    groups = k // 4
    kh = k // 2                    # half-row length (f32 load halves)

    nc = bacc.Bacc()
    x = nc.declare_dram_parameter("x", [m_core, k], F32, isOutput=False)
    w = nc.declare_dram_parameter("weight", [n, k], F32, isOutput=False)
    out = nc.declare_dram_parameter("out", [m_core, n], BF16, isOutput=True)

    with tile.TileContext(nc) as tc:
        with (
            tc.tile_pool(name="dram", bufs=1, space="DRAM") as dpool,
            tc.tile_pool(name="per", bufs=1) as perpool,
            tc.tile_pool(name="wld", bufs=2) as wldpool,
            tc.tile_pool(name="wq8", bufs=1) as wqpool,
            tc.tile_pool(name="wtp", bufs=1) as wtppool,
            tc.tile_pool(name="xld", bufs=2) as xldpool,
            tc.tile_pool(name="xu8", bufs=2) as u8pool,
            tc.tile_pool(name="cmp", bufs=1) as cpool,
            tc.tile_pool(name="xsT0", bufs=3) as xspT0pool,
            tc.tile_pool(name="xsT", bufs=2) as xspTpool,
            tc.tile_pool(name="sml", bufs=4) as spool,
            tc.tile_pool(name="ep", bufs=1) as eppool,
            tc.tile_pool(name="ps", bufs=8, space="PSUM") as pspool,
        ):
            xsp_dram = dpool.tile([m_core, k], FP8)
            wq_dram = dpool.tile([n, k], FP8)
            wamax_dram = dpool.tile([n], F32)

            # persistent SBUF
            wk2 = perpool.tile([P, kp_tiles, 2, n], FP8)       # 64KB/part
            swb = perpool.tile([P, n], F32)                    # 8KB/part
            sx4 = perpool.tile([P, m_tiles], F32)

            xspT = {}  # m -> [P, kp_tiles, 2, P] fp8 tile

            # ---------------- W path ----------------
            def w_quant(j):
                wt = wldpool.tile([P, k], F32, tag="wt")
                nc.sync.dma_start(wt, w[P * j : P * (j + 1), :])
                amax = spool.tile([P, 1], F32, tag="wamax")
                nc.vector.tensor_reduce(
                    amax, wt, axis=AX, op=OP.max, apply_absolute_value=True
                )
                g = spool.tile([P, 1], F32, tag="wg")
                nc.vector.reciprocal(g, amax)
                nc.vector.tensor_scalar_mul(g, g, 224.0)
                u8 = wqpool.tile([P, k], FP8, tag="wu8")
                nc.scalar.activation(u8, wt, AF.Copy, scale=g)
                nc.sync.dma_start(wamax_dram[P * j : P * (j + 1)], amax)
                nc.sync.dma_start(wq_dram[P * j : P * (j + 1), :], u8)

            def w_band(ns):
                # transpose band [512, 2048]u16 -> [128, 16, 512], by k-halves
                kp2 = kp_tiles // 2
                for kb in range(2):
                    wtp = wtppool.tile([P, kp2, N_SLICE], U16, tag="wtp")
                    nc.sync.dma_start_transpose(
                        wtp,
                        wq_dram.bitcast(U16)[
                            N_SLICE * ns : N_SLICE * (ns + 1),
                            P * kp2 * kb : P * kp2 * (kb + 1),
                        ],
                    )
                    pk = wtp.bitcast(FP8).rearrange("p t (r o) -> p t r o", o=2)
                    for o in range(2):
                        nc.scalar.activation(
                            wk2[
                                :, kp2 * kb : kp2 * (kb + 1), o,
                                N_SLICE * ns : N_SLICE * (ns + 1),
                            ],
                            pk[:, :, :, o],
                            AF.Copy,
                        )
                # broadcast W amax into swb
                nc.sync.dma_start(
                    swb[:, N_SLICE * ns : N_SLICE * (ns + 1)],
                    wamax_dram[N_SLICE * ns : N_SLICE * (ns + 1)]
                    .unsqueeze(0)
                    .to_broadcast([P, N_SLICE]),
                )

            # ---------------- X path ----------------
            def x_tile(mt):
                xt = xldpool.tile([P, k], F32, tag="xt")
                nc.sync.dma_start(xt, x[P * mt : P * (mt + 1), :])
                amax = spool.tile([P, 1], F32, tag="xamax")
                nc.vector.tensor_reduce(
                    amax, xt, axis=AX, op=OP.max, apply_absolute_value=True
                )
                nc.vector.tensor_scalar_mul(sx4[:, mt : mt + 1], amax, SX_CONST)
                g = spool.tile([P, 1], F32, tag="xg")
                nc.vector.reciprocal(g, amax)
                nc.vector.tensor_scalar_mul(g, g, 224.0)
                u8 = u8pool.tile([P, k], FP8, tag="xu8")
                nc.scalar.activation(u8, xt, AF.Copy, scale=g)

                # ---- 2:4 selection in packed u16 domain (DVE) ----
                # SBUF-overlaid scratch: magmask holds mag then (later) the
                # byte-mask; scr holds mlo+mhi then (later) the kk flags;
                # b6mt holds the 6 pairwise compares then (later) mtmp;
                # stile holds the s partial sums then (later) the masked xsp.
                magmask = cpool.tile([P, k // 2], U16, tag="magmask")
                scr = cpool.tile([P, k], U16, tag="scr")
                b6mt = cpool.tile([P, 6, groups], U16, tag="b6mt")
                stile = cpool.tile([P, k // 2], U16, tag="stile")

                mag = magmask
                nc.vector.tensor_scalar(
                    mag, u8.bitcast(U16), 0x7F7F, None, op0=OP.bitwise_and
                )
                mlo = scr[:, : k // 2]
                nc.vector.tensor_scalar(mlo, mag, 0x00FF, None, op0=OP.bitwise_and)
                mhi = scr[:, k // 2 :]
                nc.vector.tensor_scalar(
                    mhi, mag, 8, None, op0=OP.logical_shift_right
                )
                lo = mlo.rearrange("p (g t) -> p g t", t=2)
                hi = mhi.rearrange("p (g t) -> p g t", t=2)
                e = {0: lo[:, :, 0], 1: hi[:, :, 0], 2: lo[:, :, 1], 3: hi[:, :, 1]}

                b6 = b6mt
                pairs = [(0, 1), (0, 2), (0, 3), (1, 2), (1, 3), (2, 3)]
                bidx = {}
                for pi, (i, jj) in enumerate(pairs):
                    nc.vector.tensor_tensor(b6[:, pi, :], e[i], e[jj], op=OP.is_ge)
                    bidx[(i, jj)] = pi

                def b(i, jj):
                    return b6[:, bidx[(i, jj)], :]

                kk = scr.bitcast(BF16).rearrange("p (a g) -> p a g", a=4)
                s = stile.bitcast(BF16).rearrange("p (a g) -> p a g", a=2)
                nc.vector.tensor_tensor(s[:, 0, :], b(0, 1), b(0, 2), op=OP.add)
                nc.vector.tensor_tensor(s[:, 0, :], s[:, 0, :], b(0, 3), op=OP.add)
                nc.vector.tensor_scalar(kk[:, 0, :], s[:, 0, :], 2.0, None, op0=OP.is_ge)
                nc.vector.tensor_tensor(s[:, 1, :], b(1, 2), b(1, 3), op=OP.add)
                nc.vector.tensor_tensor(s[:, 1, :], s[:, 1, :], b(0, 1), op=OP.subtract)
                nc.vector.tensor_scalar(kk[:, 1, :], s[:, 1, :], 1.0, None, op0=OP.is_ge)
                nc.vector.tensor_tensor(s[:, 0, :], b(2, 3), b(0, 2), op=OP.subtract)
                nc.vector.tensor_tensor(s[:, 0, :], s[:, 0, :], b(1, 2), op=OP.subtract)
                nc.vector.tensor_scalar(kk[:, 2, :], s[:, 0, :], 0.0, None, op0=OP.is_ge)
                nc.vector.tensor_tensor(s[:, 1, :], b(0, 3), b(1, 3), op=OP.add)
                nc.vector.tensor_tensor(s[:, 1, :], s[:, 1, :], b(2, 3), op=OP.add)
                nc.vector.tensor_scalar(kk[:, 3, :], s[:, 1, :], 1.0, None, op0=OP.is_le)

                mtmp = b6mt.bitcast(BF16).rearrange("p a g -> p a g")[:, :2, :]
                nc.vector.tensor_scalar_mul(mtmp[:, 0, :], kk[:, 0, :], 255.0)
                nc.vector.tensor_scalar_mul(mtmp[:, 1, :], kk[:, 2, :], 255.0)
                mask = magmask
                mv = mask.rearrange("p (g t) -> p g t", t=2)
                nc.vector.scalar_tensor_tensor(
                    mv[:, :, 0], kk[:, 1, :], 65280.0, mtmp[:, 0, :],
                    op0=OP.mult, op1=OP.add,
                )
                nc.vector.scalar_tensor_tensor(
                    mv[:, :, 1], kk[:, 3, :], 65280.0, mtmp[:, 1, :],
                    op0=OP.mult, op1=OP.add,
                )
                xsp = stile
                nc.vector.tensor_tensor(xsp, u8.bitcast(U16), mask, op=OP.bitwise_and)
                nc.sync.dma_start(
                    xsp_dram.bitcast(U16)[P * mt : P * (mt + 1), :], xsp
                )

                # transpose this m-row-block: [128, 2048]u16 -> [128, 16, 128]
                # (overlaid on scr, whose kk contents are dead by now)
                xtp = scr[:, : k // 2].rearrange("p (t r) -> p t r", t=kp_tiles)
                nc.sync.dma_start_transpose(
                    xtp, xsp_dram.bitcast(U16)[P * mt : P * (mt + 1), :]
                )
                pool_ = xspT0pool if mt < 3 else xspTpool
                xq = pool_.tile([P, kp_tiles, 2, P], FP8, tag="xq", name="xq")
                pk = xtp.bitcast(FP8).rearrange("p t (r o) -> p t r o", o=2)
                nc.scalar.activation(xq[:, :, 0, :], pk[:, :, :, 0], AF.Copy)
                nc.scalar.activation(xq[:, :, 1, :], pk[:, :, :, 1], AF.Copy)
                xspT[mt] = xq

            # ---------------- MM pair ----------------
            def mm_pair(mt, ns):
                ps = pspool.tile([P, N_SLICE], F32, tag="ps", name=f"ps{mt}_{ns}")
                xq = xspT[mt]
                for t in range(kp_tiles):
                    nc.tensor.matmul(
                        ps,
                        lhsT=xq[:, t, :, :],
                        rhs=wk2[:, t, :, N_SLICE * ns : N_SLICE * (ns + 1)],
                        perf_mode=mybir.MatmulPerfMode.DoubleRow,
                        start=(t == 0),
                        stop=(t == kp_tiles - 1),
                    )
                ob = eppool.tile([P, N_SLICE], F32, tag="ob")
                nc.vector.tensor_tensor(
                    ob, ps, swb[:, N_SLICE * ns : N_SLICE * (ns + 1)], op=OP.mult
                )
                ob2 = eppool.tile([P, N_SLICE], BF16, tag="ob2")
                nc.scalar.activation(ob2, ob, AF.Copy, scale=sx4[:, mt : mt + 1])
                nc.sync.dma_start(
                    out[P * mt : P * (mt + 1), N_SLICE * ns : N_SLICE * (ns + 1)],
                    ob2,
                )

            # ---------------- emission schedule ----------------
            # X0-2 first (DVE starts immediately; their xspT tiles live in a
            # dedicated pinned pool since their MM pairs are emitted last),
            # then the whole W path, then the steady-state X pipeline where
            # pairs(m, 0..3) are emitted en-bloc right after x_tile(m) — so
            # the rolling xsT pool (bufs=2) is correct by construction.
            # The PE executes by semaphore readiness, not emission position,
            # so matmuls start as soon as band 0 + one X tile are done.
            for bn in range(n_slices):
                if bn < 3:
                    x_tile(bn)
                for j in range(wt_per_band * bn, wt_per_band * (bn + 1)):
                    w_quant(j)
                w_band(bn)
            for mt in range(3, m_tiles):
                x_tile(mt)
                for ns in range(n_slices):
                    mm_pair(mt, ns)
            for mt in range(3):
                for ns in range(n_slices):
                    mm_pair(mt, ns)

    return nc


_NC = None


def make_in_maps(x: np.ndarray, weight: np.ndarray) -> list[dict]:
    x = np.ascontiguousarray(x, dtype=np.float32)
    weight = np.ascontiguousarray(weight, dtype=np.float32)
    in_maps = []
    for c in range(NCORES):
        mg, ng = c // NG, c % NG
        in_maps.append(
            {
                "x": x[mg * M_CORE : (mg + 1) * M_CORE],
                "weight": weight[ng * N_CORE : (ng + 1) * N_CORE],
            }
        )
    return in_maps


def assemble_out(results: list[dict]) -> np.ndarray:
    rows = []
    for mg in range(MG):
        blocks = [results[mg * NG + ng]["out"] for ng in range(NG)]
        rows.append(np.concatenate(blocks, axis=1))
    return np.concatenate(rows, axis=0)


def kernel(x: np.ndarray, weight: np.ndarray) -> np.ndarray:
    global _NC
    if _NC is None:
        _NC = build_nc()
        _NC.finalize()
    res = run_bass_kernel_spmd(_NC, make_in_maps(x, weight), list(range(NCORES)))
    return assemble_out(res.results)


# revision 52
# speedup vs baseline: 1.2890x; 1.1164x over previous
"""FP8 semi-sparse (2:4) activation linear — Trainium2 Bass/Tile kernel, v2.

Reference semantics:
  Wq, W_scale = rowwise fp8(e4m3fn) quant of weight      [N, K]
  Xq, X_scale = rowwise fp8(e4m3fn) quant of x           [M, K]
  Xsp         = 2:4 sparsify of Xq (keep 2 largest |.| per group of 4,
                ties -> earlier index)
  out         = (Xsp @ Wq^T) * X_scale * W_scale^T  -> bf16

v2 design (vs the v1 data-parallel kernel):
  * 2D core grid 4x2: core c=(mg*2+ng) gets x rows [mg*2048,+2048) and W rows
    [ng*2048,+2048), computes the [2048, 2048] out block.  Halves the
    replicated W work and the per-core HBM traffic vs pure DP.
  * W^T fp8 (deinterleaved DoubleRow planes) is SBUF-resident (64KB/part);
    quantized in 4 bands of 512 rows so matmuls can start after band 0.
  * m-outer software pipeline: X-path(m) on DVE runs ahead while PE drains
    matmul pairs (m, ns); keeps PE continuously busy so it p-state ramps.
  * Engine balance: DVE owns the 2:4 selection (+half of each X amax);
    ACT owns the fp8 casts + deinterleaves + psum->bf16 epilogue cast;
    Pool(gpsimd) owns W amax folding, X amax half-folds, and the bf16
    epilogue scale-by-W_amax multiply.
  * Transposes are single big XBAR instructions ([rows, 2048]u16 ->
    [128, 16, rows]) straight from DRAM staging, 3D-out form.
  * TRN fp8e4 (max 240) vs OCP e4m3fn (max 448): quantize at HALF scale
    (g = 224/amax), fold the 4x into the output scale constant.
"""

import numpy as np

import concourse.bass as bass
import concourse.mybir as mybir
import concourse.tile as tile
from concourse import bacc
from concourse.bass_utils import run_bass_kernel_spmd

P = 128
M_FULL, K_FULL, N_FULL = 8192, 4096, 4096
NCORES = 8
MG, NG = 4, 2
M_CORE = M_FULL // MG    # 2048
N_CORE = N_FULL // NG    # 2048
N_SLICE = 512

F32 = mybir.dt.float32
BF16 = mybir.dt.bfloat16
FP8 = mybir.dt.float8e4
U16 = mybir.dt.uint16

AX = mybir.AxisListType.X
OP = mybir.AluOpType
AF = mybir.ActivationFunctionType

# out = acc' * amax_w * (amax_x * 4/448^2); acc' is the matmul of halved values
SX_CONST = float(np.float32(4.0 / (448.0 * 448.0)))


U32 = mybir.dt.uint32


def build_nc(m_core=M_CORE, k=K_FULL, n=N_CORE) -> bass.Bass:
    assert m_core % P == 0 and k % (2 * P) == 0 and n % N_SLICE == 0
    m_tiles = m_core // P          # 16
    kp_tiles = k // (2 * P)        # 16 packed k-pair tiles
    n_slices = n // N_SLICE        # 4 (also W bands)
    w_tiles = n // P               # 16
    wt_per_band = w_tiles // n_slices  # 4
    groups = k // 4
    kh = k // 2                    # half-row length (f32 load halves)

    nc = bacc.Bacc()
    x = nc.declare_dram_parameter("x", [m_core, k], F32, isOutput=False)
    w = nc.declare_dram_parameter("weight", [n, k], F32, isOutput=False)
    out = nc.declare_dram_parameter("out", [m_core, n], BF16, isOutput=True)

    with tile.TileContext(nc) as tc:
        with (
            tc.tile_pool(name="dram", bufs=1, space="DRAM") as dpool,
            tc.tile_pool(name="per", bufs=1) as perpool,
            tc.tile_pool(name="wld", bufs=2) as wldpool,
            tc.tile_pool(name="wq8", bufs=1) as wqpool,
            tc.tile_pool(name="wtp", bufs=1) as wtppool,
            tc.tile_pool(name="xld", bufs=2) as xldpool,
            tc.tile_pool(name="xu8", bufs=2) as u8pool,
            tc.tile_pool(name="cmp", bufs=1) as cpool,
            tc.tile_pool(name="xsT0", bufs=3) as xspT0pool,
            tc.tile_pool(name="xsT", bufs=2) as xspTpool,
            tc.tile_pool(name="sml", bufs=4) as spool,
            tc.tile_pool(name="ep", bufs=2) as eppool,
            tc.tile_pool(name="ps", bufs=8, space="PSUM") as pspool,
        ):
            wq_dram = dpool.tile([n, k], FP8)
            wamax_dram = dpool.tile([n], F32)

            # persistent SBUF
            wk2 = perpool.tile([P, kp_tiles, 2, n], FP8)       # 64KB/part
            swb = perpool.tile([P, n], F32)                    # 8KB/part
            sx4 = perpool.tile([P, m_tiles], F32)

            xspT = {}  # m -> [P, kp_tiles, 2, P] fp8 tile

            # ---------------- W path ----------------
            def w_quant(j):
                wt = wldpool.tile([P, k], F32, tag="wt")
                nc.sync.dma_start(wt, w[P * j : P * (j + 1), :])
                amax = spool.tile([P, 1], F32, tag="wamax")
                nc.vector.tensor_reduce(
                    amax, wt, axis=AX, op=OP.max, apply_absolute_value=True
                )
                g = spool.tile([P, 1], F32, tag="wg")
                nc.vector.reciprocal(g, amax)
                nc.vector.tensor_scalar_mul(g, g, 224.0)
                u8 = wqpool.tile([P, k], FP8, tag="wu8")
                nc.scalar.activation(u8, wt, AF.Copy, scale=g)
                nc.sync.dma_start(wamax_dram[P * j : P * (j + 1)], amax)
                nc.sync.dma_start(wq_dram[P * j : P * (j + 1), :], u8)

            def w_band(ns):
                # transpose band [512, 2048]u16 -> [128, 16, 512], by k-quarters
                kp2 = kp_tiles // 4
                for kb in range(4):
                    wtp = wtppool.tile([P, kp2, N_SLICE], U16, tag="wtp")
                    nc.sync.dma_start_transpose(
                        wtp,
                        wq_dram.bitcast(U16)[
                            N_SLICE * ns : N_SLICE * (ns + 1),
                            P * kp2 * kb : P * kp2 * (kb + 1),
                        ],
                    )
                    pk = wtp.bitcast(FP8).rearrange("p t (r o) -> p t r o", o=2)
                    for o in range(2):
                        nc.scalar.activation(
                            wk2[
                                :, kp2 * kb : kp2 * (kb + 1), o,
                                N_SLICE * ns : N_SLICE * (ns + 1),
                            ],
                            pk[:, :, :, o],
                            AF.Copy,
                        )
                # broadcast W amax into swb
                nc.sync.dma_start(
                    swb[:, N_SLICE * ns : N_SLICE * (ns + 1)],
                    wamax_dram[N_SLICE * ns : N_SLICE * (ns + 1)]
                    .unsqueeze(0)
                    .to_broadcast([P, N_SLICE]),
                )

            # ---------------- X path ----------------
            def x_tile(mt):
                xt = xldpool.tile([P, k], F32, tag="xt")
                nc.sync.dma_start(xt, x[P * mt : P * (mt + 1), :])
                amax = spool.tile([P, 1], F32, tag="xamax")
                nc.vector.tensor_reduce(
                    amax, xt, axis=AX, op=OP.max, apply_absolute_value=True
                )
                nc.vector.tensor_scalar_mul(sx4[:, mt : mt + 1], amax, SX_CONST)
                g = spool.tile([P, 1], F32, tag="xg")
                nc.vector.reciprocal(g, amax)
                nc.vector.tensor_scalar_mul(g, g, 224.0)
                u8 = u8pool.tile([P, k], FP8, tag="xu8")
                nc.scalar.activation(u8, xt, AF.Copy, scale=g)

                # ---- 2:4 selection in packed u16 domain (DVE) ----
                # SBUF-overlaid scratch: magmask holds mag then (later) the
                # byte-mask; scr holds mlo+mhi then (later) the kk flags;
                # b6mt holds the 6 pairwise compares then (later) mtmp;
                # stile holds the s partial sums then (later) the masked xsp.
                magmask = cpool.tile([P, k // 2], U16, tag="magmask")
                scr = cpool.tile([P, k], U16, tag="scr")
                b6mt = cpool.tile([P, 6, groups], U16, tag="b6mt")
                stile = cpool.tile([P, k // 2], U16, tag="stile")

                mag = magmask
                nc.vector.tensor_scalar(
                    mag, u8.bitcast(U16), 0x7F7F, None, op0=OP.bitwise_and
                )
                mlo = scr[:, : k // 2]
                nc.vector.tensor_scalar(mlo, mag, 0x00FF, None, op0=OP.bitwise_and)
                mhi = scr[:, k // 2 :]
                nc.vector.tensor_scalar(
                    mhi, mag, 8, None, op0=OP.logical_shift_right
                )
                lo = mlo.rearrange("p (g t) -> p g t", t=2)
                hi = mhi.rearrange("p (g t) -> p g t", t=2)
                e = {0: lo[:, :, 0], 1: hi[:, :, 0], 2: lo[:, :, 1], 3: hi[:, :, 1]}

                b6 = b6mt
                pairs = [(0, 1), (0, 2), (0, 3), (1, 2), (1, 3), (2, 3)]
                bidx = {}
                for pi, (i, jj) in enumerate(pairs):
                    nc.vector.tensor_tensor(b6[:, pi, :], e[i], e[jj], op=OP.is_ge)
                    bidx[(i, jj)] = pi

                def b(i, jj):
                    return b6[:, bidx[(i, jj)], :]

                kk = scr.bitcast(BF16).rearrange("p (a g) -> p a g", a=4)
                s = stile.bitcast(BF16).rearrange("p (a g) -> p a g", a=2)
                nc.vector.tensor_tensor(s[:, 0, :], b(0, 1), b(0, 2), op=OP.add)
                nc.vector.tensor_tensor(s[:, 0, :], s[:, 0, :], b(0, 3), op=OP.add)
                nc.vector.tensor_scalar(kk[:, 0, :], s[:, 0, :], 2.0, None, op0=OP.is_ge)
                nc.vector.tensor_tensor(s[:, 1, :], b(1, 2), b(1, 3), op=OP.add)
                nc.vector.tensor_tensor(s[:, 1, :], s[:, 1, :], b(0, 1), op=OP.subtract)
                nc.vector.tensor_scalar(kk[:, 1, :], s[:, 1, :], 1.0, None, op0=OP.is_ge)
                nc.vector.tensor_tensor(s[:, 0, :], b(2, 3), b(0, 2), op=OP.subtract)
                nc.vector.tensor_tensor(s[:, 0, :], s[:, 0, :], b(1, 2), op=OP.subtract)
                nc.vector.tensor_scalar(kk[:, 2, :], s[:, 0, :], 0.0, None, op0=OP.is_ge)
                nc.vector.tensor_tensor(s[:, 1, :], b(0, 3), b(1, 3), op=OP.add)
                nc.vector.tensor_tensor(s[:, 1, :], s[:, 1, :], b(2, 3), op=OP.add)
                nc.vector.tensor_scalar(kk[:, 3, :], s[:, 1, :], 1.0, None, op0=OP.is_le)

                mtmp = b6mt.bitcast(BF16).rearrange("p a g -> p a g")[:, :2, :]
                nc.vector.tensor_scalar_mul(mtmp[:, 0, :], kk[:, 0, :], 255.0)
                nc.vector.tensor_scalar_mul(mtmp[:, 1, :], kk[:, 2, :], 255.0)
                mask = magmask
                mv = mask.rearrange("p (g t) -> p g t", t=2)
                nc.vector.scalar_tensor_tensor(
                    mv[:, :, 0], kk[:, 1, :], 65280.0, mtmp[:, 0, :],
                    op0=OP.mult, op1=OP.add,
                )
                nc.vector.scalar_tensor_tensor(
                    mv[:, :, 1], kk[:, 3, :], 65280.0, mtmp[:, 1, :],
                    op0=OP.mult, op1=OP.add,
                )
                xsp = stile
                nc.vector.tensor_tensor(xsp, u8.bitcast(U16), mask, op=OP.bitwise_and)

                # SBUF->SBUF transpose [128, 2048]u16 -> [128, 16, 128];
                # output overlays u8 (fully consumed by the AND above), so the
                # cmp scratch frees without waiting on the deint.
                xtp = u8.bitcast(U16).rearrange("p (t r) -> p t r", t=kp_tiles)
                nc.sync.dma_start_transpose(xtp, xsp)
                pool_ = xspT0pool if mt < 3 else xspTpool
                xq = pool_.tile([P, kp_tiles, 2, P], FP8, tag="xq", name="xq")
                pk = xtp.bitcast(FP8).rearrange("p t (r o) -> p t r o", o=2)
                nc.scalar.activation(xq[:, :, 0, :], pk[:, :, :, 0], AF.Copy)
                nc.scalar.activation(xq[:, :, 1, :], pk[:, :, :, 1], AF.Copy)
                xspT[mt] = xq

            # ---------------- MM pair ----------------
            def mm_pair(mt, ns):
                ps = pspool.tile([P, N_SLICE], F32, tag="ps", name=f"ps{mt}_{ns}")
                xq = xspT[mt]
                for t in range(kp_tiles):
                    nc.tensor.matmul(
                        ps,
                        lhsT=xq[:, t, :, :],
                        rhs=wk2[:, t, :, N_SLICE * ns : N_SLICE * (ns + 1)],
                        perf_mode=mybir.MatmulPerfMode.DoubleRow,
                        start=(t == 0),
                        stop=(t == kp_tiles - 1),
                    )
                ob = eppool.tile([P, N_SLICE], F32, tag="ob")
                nc.vector.tensor_tensor(
                    ob, ps, swb[:, N_SLICE * ns : N_SLICE * (ns + 1)], op=OP.mult
                )
                ob2 = eppool.tile([P, N_SLICE], BF16, tag="ob2")
                nc.scalar.activation(ob2, ob, AF.Copy, scale=sx4[:, mt : mt + 1])
                nc.sync.dma_start(
                    out[P * mt : P * (mt + 1), N_SLICE * ns : N_SLICE * (ns + 1)],
                    ob2,
                )

            # ---------------- emission schedule ----------------
            # X0-2 first (DVE starts immediately; their xspT tiles live in a
            # dedicated pinned pool since their MM pairs are emitted last),
            # then the whole W path, then the steady-state X pipeline where
            # pairs(m, 0..3) are emitted en-bloc right after x_tile(m) — so
            # the rolling xsT pool (bufs=2) is correct by construction.
            # The PE executes by semaphore readiness, not emission position,
            # so matmuls start as soon as band 0 + one X tile are done.
            for bn in range(n_slices):
                if bn < 3:
                    x_tile(bn)
                for j in range(wt_per_band * bn, wt_per_band * (bn + 1)):
                    w_quant(j)
                w_band(bn)
            # pairs lag one X tile so their DVE epilogue mults never stall
            # the in-order DVE queue waiting on the PE.
            x_tile(3)
            for mt in range(4, m_tiles):
                x_tile(mt)
                for ns in range(n_slices):
                    mm_pair(mt - 1, ns)
            for mt in (m_tiles - 1, 0, 1, 2):
                for ns in range(n_slices):
                    mm_pair(mt, ns)

    return nc


_NC = None


def make_in_maps(x: np.ndarray, weight: np.ndarray) -> list[dict]:
    x = np.ascontiguousarray(x, dtype=np.float32)
    weight = np.ascontiguousarray(weight, dtype=np.float32)
    in_maps = []
    for c in range(NCORES):
        mg, ng = c // NG, c % NG
        in_maps.append(
            {
                "x": x[mg * M_CORE : (mg + 1) * M_CORE],
                "weight": weight[ng * N_CORE : (ng + 1) * N_CORE],
            }
        )
    return in_maps


def assemble_out(results: list[dict]) -> np.ndarray:
    rows = []
    for mg in range(MG):
        blocks = [results[mg * NG + ng]["out"] for ng in range(NG)]
        rows.append(np.concatenate(blocks, axis=1))
    return np.concatenate(rows, axis=0)


def kernel(x: np.ndarray, weight: np.ndarray) -> np.ndarray:
    global _NC
    if _NC is None:
        _NC = build_nc()
        _NC.finalize()
    res = run_bass_kernel_spmd(_NC, make_in_maps(x, weight), list(range(NCORES)))
    return assemble_out(res.results)


# revision 58
# speedup vs baseline: 1.4456x; 1.1215x over previous
"""FP8 semi-sparse (2:4) activation linear — Trainium2 Bass/Tile kernel, v2.

Reference semantics:
  Wq, W_scale = rowwise fp8(e4m3fn) quant of weight      [N, K]
  Xq, X_scale = rowwise fp8(e4m3fn) quant of x           [M, K]
  Xsp         = 2:4 sparsify of Xq (keep 2 largest |.| per group of 4,
                ties -> earlier index)
  out         = (Xsp @ Wq^T) * X_scale * W_scale^T  -> bf16

v2 design (vs the v1 data-parallel kernel):
  * 2D core grid 4x2: core c=(mg*2+ng) gets x rows [mg*2048,+2048) and W rows
    [ng*2048,+2048), computes the [2048, 2048] out block.  Halves the
    replicated W work and the per-core HBM traffic vs pure DP.
  * W^T fp8 (deinterleaved DoubleRow planes) is SBUF-resident (64KB/part);
    quantized in 4 bands of 512 rows so matmuls can start after band 0.
  * m-outer software pipeline: X-path(m) on DVE runs ahead while PE drains
    matmul pairs (m, ns); keeps PE continuously busy so it p-state ramps.
  * Engine balance: DVE owns the 2:4 selection (+half of each X amax);
    ACT owns the fp8 casts + deinterleaves + psum->bf16 epilogue cast;
    Pool(gpsimd) owns W amax folding, X amax half-folds, and the bf16
    epilogue scale-by-W_amax multiply.
  * Transposes are single big XBAR instructions ([rows, 2048]u16 ->
    [128, 16, rows]) straight from DRAM staging, 3D-out form.
  * TRN fp8e4 (max 240) vs OCP e4m3fn (max 448): quantize at HALF scale
    (g = 224/amax), fold the 4x into the output scale constant.
"""

import numpy as np

import concourse.bass as bass
import concourse.mybir as mybir
import concourse.tile as tile
from concourse import bacc
from concourse.bass_utils import run_bass_kernel_spmd

P = 128
M_FULL, K_FULL, N_FULL = 8192, 4096, 4096
NCORES = 8
MG, NG = 4, 2
M_CORE = M_FULL // MG    # 2048
N_CORE = N_FULL // NG    # 2048
N_SLICE = 512

F32 = mybir.dt.float32
BF16 = mybir.dt.bfloat16
FP8 = mybir.dt.float8e4
U16 = mybir.dt.uint16

AX = mybir.AxisListType.X
OP = mybir.AluOpType
AF = mybir.ActivationFunctionType

# out = acc' * amax_w * (amax_x * 4/448^2); acc' is the matmul of halved values
SX_CONST = float(np.float32(4.0 / (448.0 * 448.0)))


U32 = mybir.dt.uint32


def build_nc(m_core=M_CORE, k=K_FULL, n=N_CORE) -> bass.Bass:
    assert m_core % P == 0 and k % (2 * P) == 0 and n % N_SLICE == 0
    m_tiles = m_core // P          # 16
    kp_tiles = k // (2 * P)        # 16 packed k-pair tiles
    n_slices = n // N_SLICE        # 4 (also W bands)
    w_tiles = n // P               # 16
    wt_per_band = w_tiles // n_slices  # 4
    groups = k // 4
    kh = k // 2                    # half-row length (f32 load halves)

    nc = bacc.Bacc()
    x = nc.declare_dram_parameter("x", [m_core, k], F32, isOutput=False)
    w = nc.declare_dram_parameter("weight", [n, k], F32, isOutput=False)
    out = nc.declare_dram_parameter("out", [m_core, n], BF16, isOutput=True)

    with tile.TileContext(nc) as tc:
        with (
            tc.tile_pool(name="dram", bufs=1, space="DRAM") as dpool,
            tc.tile_pool(name="per", bufs=1) as perpool,
            tc.tile_pool(name="wld", bufs=2) as wldpool,
            tc.tile_pool(name="wq8", bufs=1) as wqpool,
            tc.tile_pool(name="wtp", bufs=1) as wtppool,
            tc.tile_pool(name="xld", bufs=2) as xldpool,
            tc.tile_pool(name="xu8", bufs=2) as u8pool,
            tc.tile_pool(name="cmp", bufs=1) as cpool,
            tc.tile_pool(name="xsT", bufs=2) as xspTpool,
            tc.tile_pool(name="xrl", bufs=2) as xrlpool,
            tc.tile_pool(name="sml", bufs=4) as spool,
            tc.tile_pool(name="ep", bufs=2) as eppool,
            tc.tile_pool(name="ps", bufs=8, space="PSUM") as pspool,
        ):
            wq_dram = dpool.tile([n, k], FP8)
            xspT_dram = dpool.tile([m_tiles, P, kp_tiles, 2, P], FP8)
            wamax_dram = dpool.tile([n], F32)

            # persistent SBUF
            wk2 = perpool.tile([P, kp_tiles, 2, n], FP8)       # 64KB/part
            swb = perpool.tile([P, n], F32)                    # 8KB/part
            sx4 = perpool.tile([P, m_tiles], F32)

            xspT = {}  # m -> [P, kp_tiles, 2, P] fp8 tile

            # ---------------- W path ----------------
            def w_quant(j):
                wt = wldpool.tile([P, k], F32, tag="wt")
                nc.sync.dma_start(wt, w[P * j : P * (j + 1), :])
                amax = spool.tile([P, 1], F32, tag="wamax")
                nc.vector.tensor_reduce(
                    amax, wt, axis=AX, op=OP.max, apply_absolute_value=True
                )
                g = spool.tile([P, 1], F32, tag="wg")
                nc.vector.reciprocal(g, amax)
                nc.vector.tensor_scalar_mul(g, g, 224.0)
                u8 = wqpool.tile([P, k], FP8, tag="wu8")
                nc.scalar.activation(u8, wt, AF.Copy, scale=g)
                nc.sync.dma_start(wamax_dram[P * j : P * (j + 1)], amax)
                nc.sync.dma_start(wq_dram[P * j : P * (j + 1), :], u8)

            def w_band(ns):
                # transpose band [512, 2048]u16 -> [128, 16, 512], by k-quarters
                kp2 = kp_tiles // 4
                for kb in range(4):
                    wtp = wtppool.tile([P, kp2, N_SLICE], U16, tag="wtp")
                    nc.sync.dma_start_transpose(
                        wtp,
                        wq_dram.bitcast(U16)[
                            N_SLICE * ns : N_SLICE * (ns + 1),
                            P * kp2 * kb : P * kp2 * (kb + 1),
                        ],
                    )
                    pk = wtp.bitcast(FP8).rearrange("p t (r o) -> p t r o", o=2)
                    for o in range(2):
                        nc.scalar.activation(
                            wk2[
                                :, kp2 * kb : kp2 * (kb + 1), o,
                                N_SLICE * ns : N_SLICE * (ns + 1),
                            ],
                            pk[:, :, :, o],
                            AF.Copy,
                        )
                # broadcast W amax into swb
                nc.sync.dma_start(
                    swb[:, N_SLICE * ns : N_SLICE * (ns + 1)],
                    wamax_dram[N_SLICE * ns : N_SLICE * (ns + 1)]
                    .unsqueeze(0)
                    .to_broadcast([P, N_SLICE]),
                )

            # ---------------- X path ----------------
            def x_tile(mt):
                xt = xldpool.tile([P, k], F32, tag="xt")
                nc.sync.dma_start(xt, x[P * mt : P * (mt + 1), :])
                amax = spool.tile([P, 1], F32, tag="xamax")
                nc.vector.tensor_reduce(
                    amax, xt, axis=AX, op=OP.max, apply_absolute_value=True
                )
                nc.vector.tensor_scalar_mul(sx4[:, mt : mt + 1], amax, SX_CONST)
                g = spool.tile([P, 1], F32, tag="xg")
                nc.vector.reciprocal(g, amax)
                nc.vector.tensor_scalar_mul(g, g, 224.0)
                u8 = u8pool.tile([P, k], FP8, tag="xu8")
                nc.scalar.activation(u8, xt, AF.Copy, scale=g)

                # ---- 2:4 selection (DVE compares on bf16 planes) ----
                # SBUF-overlaid scratch: magmask holds mag then (later) the
                # byte-mask; scr holds the e0..e3 bf16 planes then the kk
                # flags; b6mt holds the 6 pairwise compares then mtmp;
                # stile holds the s partial sums then the masked xsp.
                magmask = cpool.tile([P, k // 2], U16, tag="magmask")
                scr = cpool.tile([P, k], U16, tag="scr")
                b6mt = cpool.tile([P, 6, groups], BF16, tag="b6mt")
                stile = cpool.tile([P, k // 2], U16, tag="stile")

                mag = magmask
                nc.vector.tensor_scalar(
                    mag, u8.bitcast(U16), 0x7F7F, None, op0=OP.bitwise_and
                )
                # ACT extracts the 4 byte-planes as contiguous bf16 (the
                # sign-stripped fp8 -> bf16 cast is exact and monotone, so
                # integer byte compares become bf16 compares).
                mview = mag.bitcast(FP8).rearrange("p (g f) -> p g f", f=4)
                ev = scr.bitcast(BF16).rearrange("p (a g) -> p a g", a=4)
                for i_ in range(4):
                    nc.scalar.activation(ev[:, i_, :], mview[:, :, i_], AF.Copy)
                e = {i_: ev[:, i_, :] for i_ in range(4)}

                b6 = b6mt
                pairs = [(0, 1), (0, 2), (0, 3), (1, 2), (1, 3), (2, 3)]
                bidx = {}
                for pi, (i, jj) in enumerate(pairs):
                    nc.vector.tensor_tensor(b6[:, pi, :], e[i], e[jj], op=OP.is_ge)
                    bidx[(i, jj)] = pi

                def b(i, jj):
                    return b6[:, bidx[(i, jj)], :]

                kk = scr.bitcast(BF16).rearrange("p (a g) -> p a g", a=4)
                s = stile.bitcast(BF16).rearrange("p (a g) -> p a g", a=2)
                nc.vector.tensor_tensor(s[:, 0, :], b(0, 1), b(0, 2), op=OP.add)
                nc.vector.tensor_tensor(s[:, 0, :], s[:, 0, :], b(0, 3), op=OP.add)
                nc.vector.tensor_scalar(kk[:, 0, :], s[:, 0, :], 2.0, None, op0=OP.is_ge)
                nc.vector.tensor_tensor(s[:, 1, :], b(1, 2), b(1, 3), op=OP.add)
                nc.vector.tensor_tensor(s[:, 1, :], s[:, 1, :], b(0, 1), op=OP.subtract)
                nc.vector.tensor_scalar(kk[:, 1, :], s[:, 1, :], 1.0, None, op0=OP.is_ge)
                nc.vector.tensor_tensor(s[:, 0, :], b(2, 3), b(0, 2), op=OP.subtract)
                nc.vector.tensor_tensor(s[:, 0, :], s[:, 0, :], b(1, 2), op=OP.subtract)
                nc.vector.tensor_scalar(kk[:, 2, :], s[:, 0, :], 0.0, None, op0=OP.is_ge)
                nc.vector.tensor_tensor(s[:, 1, :], b(0, 3), b(1, 3), op=OP.add)
                nc.vector.tensor_tensor(s[:, 1, :], s[:, 1, :], b(2, 3), op=OP.add)
                nc.vector.tensor_scalar(kk[:, 3, :], s[:, 1, :], 1.0, None, op0=OP.is_le)

                mtmp = b6mt.bitcast(BF16).rearrange("p a g -> p a g")[:, :2, :]
                nc.vector.tensor_scalar_mul(mtmp[:, 0, :], kk[:, 0, :], 255.0)
                nc.vector.tensor_scalar_mul(mtmp[:, 1, :], kk[:, 2, :], 255.0)
                mask = magmask
                mv = mask.rearrange("p (g t) -> p g t", t=2)
                nc.vector.scalar_tensor_tensor(
                    mv[:, :, 0], kk[:, 1, :], 65280.0, mtmp[:, 0, :],
                    op0=OP.mult, op1=OP.add,
                )
                nc.vector.scalar_tensor_tensor(
                    mv[:, :, 1], kk[:, 3, :], 65280.0, mtmp[:, 1, :],
                    op0=OP.mult, op1=OP.add,
                )
                xsp = stile
                nc.vector.tensor_tensor(xsp, u8.bitcast(U16), mask, op=OP.bitwise_and)

                # SBUF->SBUF transpose [128, 2048]u16 -> [128, 16, 128];
                # output overlays u8 (fully consumed by the AND above), so the
                # cmp scratch frees without waiting on the deint.
                xtp = u8.bitcast(U16).rearrange("p (t r) -> p t r", t=kp_tiles)
                nc.sync.dma_start_transpose(xtp, xsp)
                xq = xspTpool.tile([P, kp_tiles, 2, P], FP8, tag="xq", name="xq")
                pk = xtp.bitcast(FP8).rearrange("p t (r o) -> p t r o", o=2)
                nc.scalar.activation(xq[:, :, 0, :], pk[:, :, :, 0], AF.Copy)
                nc.scalar.activation(xq[:, :, 1, :], pk[:, :, :, 1], AF.Copy)
                # stage to DRAM: decouples the X pipeline from band readiness
                nc.sync.dma_start(xspT_dram[mt], xq)

            # ---------------- MM pair-block (4 n-slices of one m-tile) ------
            def mm_block(mt):
                xq = xrlpool.tile([P, kp_tiles, 2, P], FP8, tag="xrl", name="xrl")
                nc.sync.dma_start(xq, xspT_dram[mt])
                for ns in range(n_slices):
                    ps = pspool.tile(
                        [P, N_SLICE], F32, tag="ps", name=f"ps{mt}_{ns}"
                    )
                    for t in range(kp_tiles):
                        nc.tensor.matmul(
                            ps,
                            lhsT=xq[:, t, :, :],
                            rhs=wk2[:, t, :, N_SLICE * ns : N_SLICE * (ns + 1)],
                            perf_mode=mybir.MatmulPerfMode.DoubleRow,
                            start=(t == 0),
                            stop=(t == kp_tiles - 1),
                        )
                    ob = eppool.tile([P, N_SLICE], F32, tag="ob", name="ob")
                    nc.vector.tensor_tensor(
                        ob, ps, swb[:, N_SLICE * ns : N_SLICE * (ns + 1)], op=OP.mult
                    )
                    ob2 = eppool.tile([P, N_SLICE], BF16, tag="ob2", name="ob2")
                    nc.scalar.activation(ob2, ob, AF.Copy, scale=sx4[:, mt : mt + 1])
                    nc.sync.dma_start(
                        out[
                            P * mt : P * (mt + 1),
                            N_SLICE * ns : N_SLICE * (ns + 1),
                        ],
                        ob2,
                    )

            # ---------------- emission schedule ----------------
            # Opening: X0..X7 interleaved with 2 W tiles each (DVE alternates
            # 2:4 and W amax; ACT alternates casts; DMA streams loads).
            # MM phase: per-m pair-blocks (reload from DRAM staging) trail the
            # remaining X tiles by one so DVE epilogue mults never stall.
            wi = 0
            for mt in range(8):
                x_tile(mt)
                for _ in range(2):
                    w_quant(wi)
                    wi += 1
                    if wi % wt_per_band == 0:
                        w_band(wi // wt_per_band - 1)
            for mt in range(m_tiles):
                if 8 + mt < m_tiles:
                    x_tile(8 + mt)
                mm_block(mt)

    return nc


_NC = None


def make_in_maps(x: np.ndarray, weight: np.ndarray) -> list[dict]:
    x = np.ascontiguousarray(x, dtype=np.float32)
    weight = np.ascontiguousarray(weight, dtype=np.float32)
    in_maps = []
    for c in range(NCORES):
        mg, ng = c // NG, c % NG
        in_maps.append(
            {
                "x": x[mg * M_CORE : (mg + 1) * M_CORE],
                "weight": weight[ng * N_CORE : (ng + 1) * N_CORE],
            }
        )
    return in_maps


def assemble_out(results: list[dict]) -> np.ndarray:
    rows = []
    for mg in range(MG):
        blocks = [results[mg * NG + ng]["out"] for ng in range(NG)]
        rows.append(np.concatenate(blocks, axis=1))
    return np.concatenate(rows, axis=0)


def kernel(x: np.ndarray, weight: np.ndarray) -> np.ndarray:
    global _NC
    if _NC is None:
        _NC = build_nc()
        _NC.finalize()
    res = run_bass_kernel_spmd(_NC, make_in_maps(x, weight), list(range(NCORES)))
    return assemble_out(res.results)
